# revision 32
# baseline (speedup 1.0000x reference)
"""Trainium2 Bass kernel for nn_MultiHeadAttention_2963527434617.

Math (per token, feature dim D=1024, 16 head-groups of 64 channels):
    Q = elu(q @ wq + bq) + 1
    K = elu(k @ wk + bk) + 1
    V = v @ wv + bv
    Kc = blockwise cumsum of K over the 16 head groups (axis=heads)
    A = Q * (K / Kc) * V            (purely elementwise)
    out = A @ wc + bc

Everything is per-token, so we shard the 32768 tokens across the 8 cores
(4096 tokens each) with zero communication.  The host feeds each core its
token slice pre-transposed to feature-major [D, tok] (bf16) so the device
never has to transpose activations for the matmuls; intermediates are kept
token-major in SBUF.

Key layout trick: the host permutes the OUTPUT columns of wq/wk/wv (and the
rows of wc to match) so that projected features are ordered channel-major,
f' = dd*16 + h.  The 16 heads of one depth-channel are then 16 consecutive
elements of the free axis, and the whole head-axis cumsum collapses into a
single masked tensor_tensor_scan (segmented prefix sum) per token block —
no 15-deep add chain, and no special-casing of head 0.

The A^T needed by the output projection is produced by the DMA transpose
xbar (one descriptor per 128-token block), keeping the PE free for real
matmuls.  Elementwise work is spread across DVE (min/recip/mults), Act
(relu/exp/V copy) and Pool (elu adds, scan, output copy) so each engine
stays well below the Tensor engine's ~27 us/chunk.  Matmul operands are
bf16 with fp32 PSUM accumulation.
"""

import sys

sys.path.insert(0, "/opt/trn_rl_repo")

import numpy as np
import ml_dtypes

B, L, D_MODEL, N_HEADS = 4, 8192, 1024, 16
D_HEAD = D_MODEL // N_HEADS  # 64
N_CORES = 8
TOKENS = B * L  # 32768
TOK_PER_CORE = TOKENS // N_CORES  # 4096
P = 128
KO = D_MODEL // P  # 8 k-blocks
T_CHUNK = 256  # tokens per pipeline chunk
N_CHUNKS = TOK_PER_CORE // T_CHUNK  # 16
MB = T_CHUNK // P  # 2 token-blocks per chunk
N_HALF = 512  # matmul moving width (one fp32 PSUM bank)
NH = D_MODEL // N_HALF  # 2

_BF16 = ml_dtypes.bfloat16

# feature permutation: new column f' = dd*16 + h  <-  old column h*64 + dd
_OLD_OF_NEW = np.array(
    [(f % N_HEADS) * D_HEAD + f // N_HEADS for f in range(D_MODEL)], dtype=np.int64
)

_module_cache = {}


def _build_module(with_bias: bool, repeat: int = 1):
    import contextlib

    import concourse.mybir as mybir
    import concourse.tile as tile
    from concourse import bacc

    f32 = mybir.dt.float32
    bf16 = mybir.dt.bfloat16
    AF = mybir.ActivationFunctionType
    OP = mybir.AluOpType

    nc = bacc.Bacc("TRN2", target_bir_lowering=False, debug=False)

    xq_d = nc.dram_tensor("xq_t", (D_MODEL, TOK_PER_CORE), bf16, kind="ExternalInput")
    xk_d = nc.dram_tensor("xk_t", (D_MODEL, TOK_PER_CORE), bf16, kind="ExternalInput")
    xv_d = nc.dram_tensor("xv_t", (D_MODEL, TOK_PER_CORE), bf16, kind="ExternalInput")
    w_d = {
        name: nc.dram_tensor(name, (D_MODEL, D_MODEL), bf16, kind="ExternalInput")
        for name in ("wq", "wk", "wv", "wc")
    }
    mask_d = nc.dram_tensor("scanmask", (P, D_MODEL), f32, kind="ExternalInput")
    out_d = nc.dram_tensor("out_t", (TOK_PER_CORE, D_MODEL), f32, kind="ExternalOutput")

    b_d = {}
    if with_bias:
        # replicated across partitions for token-major free-axis adds
        for name in ("bq", "bk", "bv", "bc"):
            b_d[name] = nc.dram_tensor(
                f"{name}_rep", (P, D_MODEL), f32, kind="ExternalInput"
            )

    xq_r = xq_d.rearrange("(ko p) t -> p ko t", p=P)
    xk_r = xk_d.rearrange("(ko p) t -> p ko t", p=P)
    xv_r = xv_d.rearrange("(ko p) t -> p ko t", p=P)

    with tile.TileContext(nc) as tc:
        with (
            tc.tile_pool(name="const", bufs=1) as constp,
            tc.tile_pool(name="xin", bufs=2) as xinp,
            tc.tile_pool(name="work", bufs=2) as workp,
            tc.tile_pool(name="small", bufs=4) as smallp,
            tc.tile_pool(name="psum1", bufs=5, space="PSUM") as pp1,
            tc.tile_pool(name="psum2", bufs=3, space="PSUM") as pp2,
        ):
            # Weights resident in SBUF; loaded on the gpsimd DMA queue so the
            # first chunk's activation loads (sync queue) are not stuck
            # behind 8 MB of weights in the same FIFO.
            w_sb = {}
            for name in ("wk", "wq", "wv", "wc"):
                t = constp.tile([P, KO, D_MODEL], bf16, tag=f"{name}_sb")
                w_r = w_d[name].rearrange("(ko p) n -> p ko n", p=P)
                if name == "wk":
                    # first weight used: one DMA per ko block, alternating
                    # queues, so the ko=0 matmul only waits ~1 us for its
                    # first 256 KB instead of 4 us for half the matrix
                    for ko in range(KO):
                        eng = nc.scalar if ko % 2 == 0 else nc.gpsimd
                        eng.dma_start(t[:, ko], w_r[:, ko])
                else:
                    nc.gpsimd.dma_start(t[:], w_r)
                w_sb[name] = t

            mask_sb = constp.tile([P, D_MODEL], f32, tag="mask_sb")
            nc.scalar.dma_start(mask_sb[:], mask_d[:])

            b_sb = {}
            if with_bias:
                for name in ("bq", "bk", "bv", "bc"):
                    t = constp.tile([P, D_MODEL], f32, tag=f"{name}_sb")
                    nc.gpsimd.dma_start(t[:], b_d[name][:])
                    b_sb[name] = t

            def proj(x_t, w, dst, bias, elu=True, qv_with=None):
                """dst[:, mb, :] (token-major [P, MB, D] bf16) = act(x @ W + b).

                Loop order mb -> ko -> nh: the stationary operand (a 128x128
                token block of x^T) is reused for both n-halves; the two
                accumulating PSUM tiles live across the ko loop.

                With qv_with=Q (the V projection), the epilogue is instead
                the fused multiply dst = Q * (x @ W + b), reading the
                projection straight out of PSUM.
                """
                for mb in range(MB):
                    # nh halves interleaved: consecutive matmuls target
                    # alternating PSUM banks, so none depends on the previous
                    # one and the PE pipeline never bubbles.
                    pss = [
                        pp1.tile([P, N_HALF], f32, tag="ps1", name=f"ps1_{mb}_{i}")
                        for i in range(NH)
                    ]
                    for ko in range(KO):
                        for nh in range(NH):
                            nc.tensor.matmul(
                                pss[nh][:],
                                lhsT=x_t[:, ko, mb * P : (mb + 1) * P],
                                rhs=w[:, ko, nh * N_HALF : (nh + 1) * N_HALF],
                                start=(ko == 0),
                                stop=(ko == KO - 1),
                            )
                    for nh in range(NH):
                        src = pss[nh][:]
                        if bias is not None:
                            tmp = smallp.tile([P, N_HALF], f32, tag="btmp")
                            nc.vector.tensor_tensor(
                                tmp[:],
                                pss[nh][:],
                                bias[:, nh * N_HALF : (nh + 1) * N_HALF],
                                OP.add,
                            )
                            src = tmp[:]
                        nhs = slice(nh * N_HALF, (nh + 1) * N_HALF)
                        dslice = dst[:, mb, nhs]
                        if elu:
                            # elu(x)+1 == relu(x) + exp(min(x, 0))  (exact).
                            # PSUM readers (min, relu) on DVE/Act — GPSIMD
                            # cannot access PSUM; the SBUF-only add goes to
                            # the otherwise idle Pool engine.
                            e = smallp.tile([P, N_HALF], bf16, tag="e")
                            nc.vector.tensor_scalar_min(e[:], src, 0.0)
                            nc.scalar.activation(dslice, src, AF.Relu)
                            nc.scalar.activation(e[:], e[:], AF.Exp)
                            nc.gpsimd.tensor_tensor(dslice, dslice, e[:], OP.add)
                        elif qv_with is not None:
                            nc.vector.tensor_tensor(
                                dslice, qv_with[:, mb, nhs], src, OP.mult
                            )
                        else:
                            nc.scalar.activation(dslice, src, AF.Copy)

            def ratio_block(K, Kc, mb):
                # Segmented prefix sum over the 16 heads: features are
                # channel-major (f' = dd*16 + h), so each depth channel's 16
                # heads are consecutive and one masked scan does the cumsum:
                #   state = mask[f]*state + K[f];  mask = 0 at f % 16 == 0.
                # (TensorTensorScanArith only exists on DVE.)  Issued after
                # the Q projection so the Q-PSUM-freeing min ops sit ahead
                # of this chain in the DVE queue.
                nc.vector.tensor_tensor_scan(
                    Kc[:, mb, :], mask_sb[:], K[:, mb, :], 0.0, OP.mult, OP.add
                )
                # Kc := 1/Kc  (Kc strictly positive: sums of elu(x)+1 > 0)
                nc.vector.reciprocal_approx_fast(Kc[:, mb], Kc[:, mb])
                # K := K * (1/Kc) — the per-head attention ratio
                nc.vector.tensor_tensor(K[:, mb], K[:, mb], Kc[:, mb], OP.mult)

            def out_block(t0, AT, mb):
                """Output projection for one 128-token block: out = A @ wc,
                token-major: lhsT = feature-major A^T block (stationary,
                reused for both n-halves), rhs = wc rows."""
                pss = [
                    pp2.tile([P, N_HALF], f32, tag="ps2", name=f"ps2_{t0}_{mb}_{i}")
                    for i in range(NH)
                ]
                for ko in range(KO):
                    for nh in range(NH):
                        nc.tensor.matmul(
                            pss[nh][:],
                            lhsT=AT[:, ko, mb * P : (mb + 1) * P],
                            rhs=w_sb["wc"][:, ko, nh * N_HALF : (nh + 1) * N_HALF],
                            start=(ko == 0),
                            stop=(ko == KO - 1),
                        )
                for nh in range(NH):
                    ot = smallp.tile([P, N_HALF], f32, tag="osb")
                    if with_bias:
                        nc.scalar.activation(ot[:], pss[nh][:], AF.Copy)
                        nc.vector.tensor_tensor(
                            ot[:],
                            ot[:],
                            b_sb["bc"][:, nh * N_HALF : (nh + 1) * N_HALF],
                            OP.add,
                        )
                    else:
                        nc.scalar.activation(ot[:], pss[nh][:], AF.Copy)
                    nc.scalar.dma_start(
                        out_d[
                            t0 + mb * P : t0 + (mb + 1) * P,
                            nh * N_HALF : (nh + 1) * N_HALF,
                        ],
                        ot[:],
                    )

            def chunk_body(c, pending):
                """Projections + elementwise for chunk c; the output
                projection for chunk c-1 (`pending`) is interleaved after
                the V projection, so its A^T operand was transposed a full
                chunk earlier and the PE never waits on the elementwise
                chain."""
                t0 = c * T_CHUNK
                xk_t = xinp.tile([P, KO, T_CHUNK], bf16, tag="xk")
                nc.sync.dma_start(xk_t[:], xk_r[:, :, t0 : t0 + T_CHUNK])
                xq_t = xinp.tile([P, KO, T_CHUNK], bf16, tag="xq")
                nc.sync.dma_start(xq_t[:], xq_r[:, :, t0 : t0 + T_CHUNK])
                xv_t = xinp.tile([P, KO, T_CHUNK], bf16, tag="xv")
                nc.sync.dma_start(xv_t[:], xv_r[:, :, t0 : t0 + T_CHUNK])

                Q = workp.tile([P, MB, D_MODEL], bf16, tag="Q")
                K = workp.tile([P, MB, D_MODEL], bf16, tag="K")
                Kc = workp.tile([P, MB, D_MODEL], f32, tag="Kc")
                A = workp.tile([P, MB, D_MODEL], bf16, tag="A")
                AT = workp.tile([P, KO, MB * P], bf16, tag="AT")

                # K first: the scan -> recip -> ratio chain (DVE, issued
                # after Q's epilogue) completes while the V projection and
                # the previous chunk's output projection occupy the PE.
                proj(xk_t, w_sb["wk"], K, b_sb.get("bk"))
                proj(xq_t, w_sb["wq"], Q, b_sb.get("bq"))
                for mb in range(MB):
                    ratio_block(K, Kc, mb)

                # Previous chunk's output projection BEFORE the V projection:
                # its PSUM->SBUF copies then clear the Act queue mid-chunk
                # instead of blocking the next chunk's epilogues at the
                # boundary.
                if pending is not None:
                    for mb in range(MB):
                        out_block(*pending, mb)

                # V projection epilogue writes A = Q * V straight from PSUM
                proj(xv_t, w_sb["wv"], A, b_sb.get("bv"), elu=False, qv_with=Q)

                for mb in range(MB):
                    # A = (Q*V) * ratio, then feature-major via the DMA xbar
                    # (one descriptor per 128-token block; 0 PE cycles)
                    nc.vector.tensor_tensor(A[:, mb], A[:, mb], K[:, mb], OP.mult)
                    nc.sync.dma_start_transpose(
                        AT[:, :, mb * P : (mb + 1) * P], A[:, mb, :]
                    )
                return (t0, AT)

            repeat_ctx = (
                tc.For_i(0, repeat, 1) if repeat > 1 else contextlib.nullcontext()
            )
            with repeat_ctx:
                pending = None
                for c in range(N_CHUNKS):
                    pending = chunk_body(c, pending)
                for mb in range(MB):
                    out_block(*pending, mb)

    nc.compile()
    return nc


def _get_module(with_bias: bool, repeat: int = 1):
    key = (bool(with_bias), repeat)
    if key not in _module_cache:
        _module_cache[key] = _build_module(*key)
    return _module_cache[key]


def _prepare_in_maps(v, k, q, wq_w, wq_b, wk_w, wk_b, wv_w, wv_b, wc_w, wc_b):
    with_bias = any(np.any(np.asarray(b)) for b in (wq_b, wk_b, wv_b, wc_b))

    q2 = np.asarray(q, dtype=np.float32).reshape(TOKENS, D_MODEL)
    k2 = np.asarray(k, dtype=np.float32).reshape(TOKENS, D_MODEL)
    v2 = np.asarray(v, dtype=np.float32).reshape(TOKENS, D_MODEL)

    # channel-major feature permutation (see module docstring)
    w16 = {
        "wq": np.ascontiguousarray(
            np.asarray(wq_w, np.float32)[:, _OLD_OF_NEW]
        ).astype(_BF16),
        "wk": np.ascontiguousarray(
            np.asarray(wk_w, np.float32)[:, _OLD_OF_NEW]
        ).astype(_BF16),
        "wv": np.ascontiguousarray(
            np.asarray(wv_w, np.float32)[:, _OLD_OF_NEW]
        ).astype(_BF16),
        "wc": np.ascontiguousarray(
            np.asarray(wc_w, np.float32)[_OLD_OF_NEW, :]
        ).astype(_BF16),
    }

    maskrow = np.where(np.arange(D_MODEL) % N_HEADS == 0, 0.0, 1.0).astype(np.float32)
    mask = np.ascontiguousarray(np.broadcast_to(maskrow, (P, D_MODEL)))

    bias_maps = {}
    if with_bias:
        bias_maps = {
            "bq_rep": np.ascontiguousarray(
                np.broadcast_to(
                    np.asarray(wq_b, np.float32)[_OLD_OF_NEW], (P, D_MODEL)
                )
            ),
            "bk_rep": np.ascontiguousarray(
                np.broadcast_to(
                    np.asarray(wk_b, np.float32)[_OLD_OF_NEW], (P, D_MODEL)
                )
            ),
            "bv_rep": np.ascontiguousarray(
                np.broadcast_to(
                    np.asarray(wv_b, np.float32)[_OLD_OF_NEW], (P, D_MODEL)
                )
            ),
            "bc_rep": np.ascontiguousarray(
                np.broadcast_to(np.asarray(wc_b, np.float32), (P, D_MODEL))
            ),
        }

    in_maps = []
    for c in range(N_CORES):
        s = slice(c * TOK_PER_CORE, (c + 1) * TOK_PER_CORE)
        m = {
            "xq_t": np.ascontiguousarray(q2[s].T).astype(_BF16),
            "xk_t": np.ascontiguousarray(k2[s].T).astype(_BF16),
            "xv_t": np.ascontiguousarray(v2[s].T).astype(_BF16),
            "scanmask": mask,
            **w16,
            **bias_maps,
        }
        in_maps.append(m)
    return in_maps, with_bias


def _assemble(results):
    out = np.empty((TOKENS, D_MODEL), np.float32)
    for c in range(N_CORES):
        out[c * TOK_PER_CORE : (c + 1) * TOK_PER_CORE] = results[c]["out_t"]
    return out.reshape(B, L, D_MODEL)


def run_kernel_raw(trace=False, **inputs):
    """Run on the 8 NeuronCores; returns (output, BassKernelResults)."""
    from concourse.bass_utils import run_bass_kernel_spmd

    in_maps, with_bias = _prepare_in_maps(**inputs)
    nc = _get_module(with_bias)
    res = run_bass_kernel_spmd(nc, in_maps, core_ids=list(range(N_CORES)), trace=trace)
    return _assemble(res.results), res


def kernel(**inputs):
    out, _ = run_kernel_raw(trace=False, **inputs)
    return out


# revision 40
# speedup vs baseline: 8.5411x; 8.5411x over previous
"""Trainium2 Bass kernel for nn_MultiHeadAttention_2963527434617.

Math (per token, feature dim D=1024, 16 head-groups of 64 channels):
    Q = elu(q @ wq + bq) + 1
    K = elu(k @ wk + bk) + 1
    V = v @ wv + bv
    Kc = blockwise cumsum of K over the 16 head groups (axis=heads)
    A = Q * (K / Kc) * V            (purely elementwise)
    out = A @ wc + bc

Everything is per-token, so we shard the 32768 tokens across the 8 cores
(4096 tokens each) with zero communication.  The host feeds each core its
token slice pre-transposed to feature-major [D, tok] (bf16) so the device
never has to transpose activations for the matmuls; intermediates are kept
token-major in SBUF.

Key layout trick: the host permutes the OUTPUT columns of wq/wk/wv (and the
rows of wc to match) so that projected features are ordered channel-major,
f' = dd*16 + h.  The 16 heads of one depth-channel are then 16 consecutive
elements of the free axis, and the whole head-axis cumsum collapses into a
single masked tensor_tensor_scan (segmented prefix sum) per token block —
no 15-deep add chain, and no special-casing of head 0.

The A^T needed by the output projection is produced by the DMA transpose
xbar (one descriptor per 128-token block), keeping the PE free for real
matmuls.  Elementwise work is spread across DVE (min/recip/mults), Act
(relu/exp/V copy) and Pool (elu adds, scan, output copy) so each engine
stays well below the Tensor engine's ~27 us/chunk.  Matmul operands are
bf16 with fp32 PSUM accumulation.
"""

import sys

sys.path.insert(0, "/opt/trn_rl_repo")

import numpy as np
import ml_dtypes

B, L, D_MODEL, N_HEADS = 4, 8192, 1024, 16
D_HEAD = D_MODEL // N_HEADS  # 64
N_CORES = 8
TOKENS = B * L  # 32768
TOK_PER_CORE = TOKENS // N_CORES  # 4096
P = 128
KO = D_MODEL // P  # 8 k-blocks
T_CHUNK = 256  # tokens per pipeline chunk
N_CHUNKS = TOK_PER_CORE // T_CHUNK  # 16
MB = T_CHUNK // P  # 2 token-blocks per chunk
N_HALF = 512  # matmul moving width (one fp32 PSUM bank)
NH = D_MODEL // N_HALF  # 2

_BF16 = ml_dtypes.bfloat16

# How A^T is produced for the output projection:
#   "dma" — DMA transpose xbar (0 PE cycles; cost model says ~0.9 us/chunk)
#   "pe"  — PE transpose via identity matmul + copy (adds ~0.85 us/chunk PE)
import os

TRANSPOSE_MODE = os.environ.get("KERNEL_TRANSPOSE", "pe")

# feature permutation: new column f' = dd*16 + h  <-  old column h*64 + dd
_OLD_OF_NEW = np.array(
    [(f % N_HEADS) * D_HEAD + f // N_HEADS for f in range(D_MODEL)], dtype=np.int64
)

_module_cache = {}


def _build_module(with_bias: bool, repeat: int = 1):
    import contextlib

    import concourse.mybir as mybir
    import concourse.tile as tile
    from concourse import bacc
    from concourse.masks import make_identity

    f32 = mybir.dt.float32
    bf16 = mybir.dt.bfloat16
    AF = mybir.ActivationFunctionType
    OP = mybir.AluOpType
    pe_transpose = TRANSPOSE_MODE == "pe"

    nc = bacc.Bacc("TRN2", target_bir_lowering=False, debug=False)

    xq_d = nc.dram_tensor("xq_t", (D_MODEL, TOK_PER_CORE), bf16, kind="ExternalInput")
    xk_d = nc.dram_tensor("xk_t", (D_MODEL, TOK_PER_CORE), bf16, kind="ExternalInput")
    xv_d = nc.dram_tensor("xv_t", (D_MODEL, TOK_PER_CORE), bf16, kind="ExternalInput")
    w_d = {
        name: nc.dram_tensor(name, (D_MODEL, D_MODEL), bf16, kind="ExternalInput")
        for name in ("wq", "wk", "wv", "wc")
    }
    mask_d = nc.dram_tensor("scanmask", (P, D_MODEL), f32, kind="ExternalInput")
    out_d = nc.dram_tensor("out_t", (TOK_PER_CORE, D_MODEL), f32, kind="ExternalOutput")

    b_d = {}
    if with_bias:
        # replicated across partitions for token-major free-axis adds
        for name in ("bq", "bk", "bv", "bc"):
            b_d[name] = nc.dram_tensor(
                f"{name}_rep", (P, D_MODEL), f32, kind="ExternalInput"
            )

    xq_r = xq_d.rearrange("(ko p) t -> p ko t", p=P)
    xk_r = xk_d.rearrange("(ko p) t -> p ko t", p=P)
    xv_r = xv_d.rearrange("(ko p) t -> p ko t", p=P)

    with tile.TileContext(nc) as tc:
        with (
            tc.tile_pool(name="const", bufs=1) as constp,
            tc.tile_pool(name="xin", bufs=2) as xinp,
            tc.tile_pool(name="work", bufs=2) as workp,
            tc.tile_pool(name="small", bufs=4) as smallp,
            tc.tile_pool(name="psum1", bufs=4 if pe_transpose else 5, space="PSUM") as pp1,
            tc.tile_pool(name="psum2", bufs=2 if pe_transpose else 3, space="PSUM") as pp2,
            tc.tile_pool(name="psumT", bufs=2, space="PSUM") as ppT,
        ):
            # Weights resident in SBUF; loaded on the gpsimd DMA queue so the
            # first chunk's activation loads (sync queue) are not stuck
            # behind 8 MB of weights in the same FIFO.
            w_sb = {}
            for name in ("wk", "wq", "wv", "wc"):
                t = constp.tile([P, KO, D_MODEL], bf16, tag=f"{name}_sb")
                w_r = w_d[name].rearrange("(ko p) n -> p ko n", p=P)
                if name == "wk":
                    # first weight used: one DMA per ko block, alternating
                    # queues, so the ko=0 matmul only waits ~1 us for its
                    # first 256 KB instead of 4 us for half the matrix
                    for ko in range(KO):
                        eng = nc.scalar if ko % 2 == 0 else nc.gpsimd
                        eng.dma_start(t[:, ko], w_r[:, ko])
                else:
                    nc.gpsimd.dma_start(t[:], w_r)
                w_sb[name] = t

            mask_sb = constp.tile([P, D_MODEL], f32, tag="mask_sb")
            nc.scalar.dma_start(mask_sb[:], mask_d[:])

            ident = None
            if pe_transpose:
                ident = constp.tile([P, P], bf16, tag="ident")
                make_identity(nc, ident)

            b_sb = {}
            if with_bias:
                for name in ("bq", "bk", "bv", "bc"):
                    t = constp.tile([P, D_MODEL], f32, tag=f"{name}_sb")
                    nc.gpsimd.dma_start(t[:], b_d[name][:])
                    b_sb[name] = t

            def proj(x_t, w, dst, bias, elu=True, qv_with=None):
                """dst[:, mb, :] (token-major [P, MB, D] bf16) = act(x @ W + b).

                Loop order mb -> ko -> nh: the stationary operand (a 128x128
                token block of x^T) is reused for both n-halves; the two
                accumulating PSUM tiles live across the ko loop.

                With qv_with=Q (the V projection), the epilogue is instead
                the fused multiply dst = Q * (x @ W + b), reading the
                projection straight out of PSUM.
                """
                for mb in range(MB):
                    # nh halves interleaved: consecutive matmuls target
                    # alternating PSUM banks, so none depends on the previous
                    # one and the PE pipeline never bubbles.
                    pss = [
                        pp1.tile([P, N_HALF], f32, tag="ps1", name=f"ps1_{mb}_{i}")
                        for i in range(NH)
                    ]
                    for ko in range(KO):
                        for nh in range(NH):
                            nc.tensor.matmul(
                                pss[nh][:],
                                lhsT=x_t[:, ko, mb * P : (mb + 1) * P],
                                rhs=w[:, ko, nh * N_HALF : (nh + 1) * N_HALF],
                                start=(ko == 0),
                                stop=(ko == KO - 1),
                            )
                    for nh in range(NH):
                        src = pss[nh][:]
                        if bias is not None:
                            tmp = smallp.tile([P, N_HALF], f32, tag="btmp")
                            nc.vector.tensor_tensor(
                                tmp[:],
                                pss[nh][:],
                                bias[:, nh * N_HALF : (nh + 1) * N_HALF],
                                OP.add,
                            )
                            src = tmp[:]
                        nhs = slice(nh * N_HALF, (nh + 1) * N_HALF)
                        dslice = dst[:, mb, nhs]
                        if elu:
                            # elu(x)+1 == relu(x) + exp(min(x, 0))  (exact).
                            # PSUM readers (min, relu) on DVE/Act — GPSIMD
                            # cannot access PSUM; the SBUF-only add goes to
                            # the otherwise idle Pool engine.
                            e = smallp.tile([P, N_HALF], bf16, tag="e")
                            nc.vector.tensor_scalar_min(e[:], src, 0.0)
                            nc.scalar.activation(dslice, src, AF.Relu)
                            nc.scalar.activation(e[:], e[:], AF.Exp)
                            nc.gpsimd.tensor_tensor(dslice, dslice, e[:], OP.add)
                        elif qv_with is not None:
                            nc.vector.tensor_tensor(
                                dslice, qv_with[:, mb, nhs], src, OP.mult
                            )
                        else:
                            nc.scalar.activation(dslice, src, AF.Copy)

            def ratio_block(K, Kc, mb):
                # Segmented prefix sum over the 16 heads: features are
                # channel-major (f' = dd*16 + h), so each depth channel's 16
                # heads are consecutive and one masked scan does the cumsum:
                #   state = mask[f]*state + K[f];  mask = 0 at f % 16 == 0.
                # (TensorTensorScanArith only exists on DVE.)  Issued after
                # the Q projection so the Q-PSUM-freeing min ops sit ahead
                # of this chain in the DVE queue.
                nc.vector.tensor_tensor_scan(
                    Kc[:, mb, :], mask_sb[:], K[:, mb, :], 0.0, OP.mult, OP.add
                )
                # Kc := 1/Kc  (Kc strictly positive: sums of elu(x)+1 > 0)
                nc.vector.reciprocal_approx_fast(Kc[:, mb], Kc[:, mb])
                # K := K * (1/Kc) — the per-head attention ratio
                nc.vector.tensor_tensor(K[:, mb], K[:, mb], Kc[:, mb], OP.mult)

            def out_block(t0, AT, mb):
                """Output projection for one 128-token block: out = A @ wc,
                token-major: lhsT = feature-major A^T block (stationary,
                reused for both n-halves), rhs = wc rows."""
                pss = [
                    pp2.tile([P, N_HALF], f32, tag="ps2", name=f"ps2_{t0}_{mb}_{i}")
                    for i in range(NH)
                ]
                for ko in range(KO):
                    for nh in range(NH):
                        nc.tensor.matmul(
                            pss[nh][:],
                            lhsT=AT[:, ko, mb * P : (mb + 1) * P],
                            rhs=w_sb["wc"][:, ko, nh * N_HALF : (nh + 1) * N_HALF],
                            start=(ko == 0),
                            stop=(ko == KO - 1),
                        )
                for nh in range(NH):
                    ot = smallp.tile([P, N_HALF], f32, tag="osb")
                    if with_bias:
                        nc.scalar.activation(ot[:], pss[nh][:], AF.Copy)
                        nc.vector.tensor_tensor(
                            ot[:],
                            ot[:],
                            b_sb["bc"][:, nh * N_HALF : (nh + 1) * N_HALF],
                            OP.add,
                        )
                    else:
                        nc.scalar.activation(ot[:], pss[nh][:], AF.Copy)
                    nc.scalar.dma_start(
                        out_d[
                            t0 + mb * P : t0 + (mb + 1) * P,
                            nh * N_HALF : (nh + 1) * N_HALF,
                        ],
                        ot[:],
                    )

            def pe_transpose_block(A, AT):
                """A -> AT on the PE via identity matmuls; runs a full chunk
                after A was produced, so it never waits on the elementwise
                chain.  PSUM->SBUF copies alternate DVE/Act."""
                i = 0
                for fb in range(KO):
                    for mb in range(MB):
                        pt = ppT.tile([P, P], bf16, tag="pT")
                        nc.tensor.transpose(
                            pt[:], A[:, mb, fb * P : (fb + 1) * P], ident[:]
                        )
                        if i % 2 == 0:
                            nc.vector.tensor_copy(
                                AT[:, fb, mb * P : (mb + 1) * P], pt[:]
                            )
                        else:
                            nc.scalar.activation(
                                AT[:, fb, mb * P : (mb + 1) * P], pt[:], AF.Copy
                            )
                        i += 1

            def chunk_body(c, pending):
                """Projections + elementwise for chunk c; the output
                projection for chunk c-1 (`pending`) is interleaved before
                the V projection, so its A^T operand was produced a full
                chunk earlier and the PE never waits on the elementwise
                chain."""
                t0 = c * T_CHUNK
                xk_t = xinp.tile([P, KO, T_CHUNK], bf16, tag="xk")
                nc.sync.dma_start(xk_t[:], xk_r[:, :, t0 : t0 + T_CHUNK])
                xq_t = xinp.tile([P, KO, T_CHUNK], bf16, tag="xq")
                nc.sync.dma_start(xq_t[:], xq_r[:, :, t0 : t0 + T_CHUNK])
                xv_t = xinp.tile([P, KO, T_CHUNK], bf16, tag="xv")
                nc.sync.dma_start(xv_t[:], xv_r[:, :, t0 : t0 + T_CHUNK])

                Q = workp.tile([P, MB, D_MODEL], bf16, tag="Q")
                K = workp.tile([P, MB, D_MODEL], bf16, tag="K")
                Kc = workp.tile([P, MB, D_MODEL], f32, tag="Kc")
                A = workp.tile([P, MB, D_MODEL], bf16, tag="A")
                AT = workp.tile([P, KO, MB * P], bf16, tag="AT")

                # K first: the scan -> recip -> ratio chain (DVE, issued
                # after Q's epilogue) completes while the V projection and
                # the previous chunk's output projection occupy the PE.
                proj(xk_t, w_sb["wk"], K, b_sb.get("bk"))
                proj(xq_t, w_sb["wq"], Q, b_sb.get("bq"))
                for mb in range(MB):
                    ratio_block(K, Kc, mb)

                # Previous chunk's transpose + output projection BEFORE the
                # V projection: the PSUM->SBUF copies then clear the engine
                # queues mid-chunk instead of blocking the next chunk's
                # epilogues at the boundary.
                if pending is not None:
                    pt0, pA, pAT = pending
                    if pe_transpose:
                        pe_transpose_block(pA, pAT)
                    for mb in range(MB):
                        out_block(pt0, pAT, mb)

                # V projection epilogue writes A = Q * V straight from PSUM
                proj(xv_t, w_sb["wv"], A, b_sb.get("bv"), elu=False, qv_with=Q)

                for mb in range(MB):
                    # A = (Q*V) * ratio
                    nc.vector.tensor_tensor(A[:, mb], A[:, mb], K[:, mb], OP.mult)
                    if not pe_transpose:
                        # feature-major via the DMA xbar (0 PE cycles)
                        nc.sync.dma_start_transpose(
                            AT[:, :, mb * P : (mb + 1) * P], A[:, mb, :]
                        )
                return (t0, A, AT)

            repeat_ctx = (
                tc.For_i(0, repeat, 1) if repeat > 1 else contextlib.nullcontext()
            )
            with repeat_ctx:
                pending = None
                for c in range(N_CHUNKS):
                    pending = chunk_body(c, pending)
                pt0, pA, pAT = pending
                if pe_transpose:
                    pe_transpose_block(pA, pAT)
                for mb in range(MB):
                    out_block(pt0, pAT, mb)

    nc.compile()
    return nc


def _get_module(with_bias: bool, repeat: int = 1):
    key = (bool(with_bias), repeat, TRANSPOSE_MODE)
    if key not in _module_cache:
        _module_cache[key] = _build_module(bool(with_bias), repeat)
    return _module_cache[key]


def _prepare_in_maps(v, k, q, wq_w, wq_b, wk_w, wk_b, wv_w, wv_b, wc_w, wc_b):
    with_bias = any(np.any(np.asarray(b)) for b in (wq_b, wk_b, wv_b, wc_b))

    q2 = np.asarray(q, dtype=np.float32).reshape(TOKENS, D_MODEL)
    k2 = np.asarray(k, dtype=np.float32).reshape(TOKENS, D_MODEL)
    v2 = np.asarray(v, dtype=np.float32).reshape(TOKENS, D_MODEL)

    # channel-major feature permutation (see module docstring)
    w16 = {
        "wq": np.ascontiguousarray(
            np.asarray(wq_w, np.float32)[:, _OLD_OF_NEW]
        ).astype(_BF16),
        "wk": np.ascontiguousarray(
            np.asarray(wk_w, np.float32)[:, _OLD_OF_NEW]
        ).astype(_BF16),
        "wv": np.ascontiguousarray(
            np.asarray(wv_w, np.float32)[:, _OLD_OF_NEW]
        ).astype(_BF16),
        "wc": np.ascontiguousarray(
            np.asarray(wc_w, np.float32)[_OLD_OF_NEW, :]
        ).astype(_BF16),
    }

    maskrow = np.where(np.arange(D_MODEL) % N_HEADS == 0, 0.0, 1.0).astype(np.float32)
    mask = np.ascontiguousarray(np.broadcast_to(maskrow, (P, D_MODEL)))

    bias_maps = {}
    if with_bias:
        bias_maps = {
            "bq_rep": np.ascontiguousarray(
                np.broadcast_to(
                    np.asarray(wq_b, np.float32)[_OLD_OF_NEW], (P, D_MODEL)
                )
            ),
            "bk_rep": np.ascontiguousarray(
                np.broadcast_to(
                    np.asarray(wk_b, np.float32)[_OLD_OF_NEW], (P, D_MODEL)
                )
            ),
            "bv_rep": np.ascontiguousarray(
                np.broadcast_to(
                    np.asarray(wv_b, np.float32)[_OLD_OF_NEW], (P, D_MODEL)
                )
            ),
            "bc_rep": np.ascontiguousarray(
                np.broadcast_to(np.asarray(wc_b, np.float32), (P, D_MODEL))
            ),
        }

    in_maps = []
    for c in range(N_CORES):
        s = slice(c * TOK_PER_CORE, (c + 1) * TOK_PER_CORE)
        m = {
            "xq_t": np.ascontiguousarray(q2[s].T).astype(_BF16),
            "xk_t": np.ascontiguousarray(k2[s].T).astype(_BF16),
            "xv_t": np.ascontiguousarray(v2[s].T).astype(_BF16),
            "scanmask": mask,
            **w16,
            **bias_maps,
        }
        in_maps.append(m)
    return in_maps, with_bias


def _assemble(results):
    out = np.empty((TOKENS, D_MODEL), np.float32)
    for c in range(N_CORES):
        out[c * TOK_PER_CORE : (c + 1) * TOK_PER_CORE] = results[c]["out_t"]
    return out.reshape(B, L, D_MODEL)


def run_kernel_raw(trace=False, **inputs):
    """Run on the 8 NeuronCores; returns (output, BassKernelResults)."""
    from concourse.bass_utils import run_bass_kernel_spmd

    in_maps, with_bias = _prepare_in_maps(**inputs)
    nc = _get_module(with_bias)
    res = run_bass_kernel_spmd(nc, in_maps, core_ids=list(range(N_CORES)), trace=trace)
    return _assemble(res.results), res


def kernel(**inputs):
    out, _ = run_kernel_raw(trace=False, **inputs)
    return out


# revision 50
# speedup vs baseline: 8.5572x; 1.0019x over previous
"""Trainium2 Bass kernel for nn_MultiHeadAttention_2963527434617.

Math (per token, feature dim D=1024, 16 head-groups of 64 channels):
    Q = elu(q @ wq + bq) + 1
    K = elu(k @ wk + bk) + 1
    V = v @ wv + bv
    Kc = blockwise cumsum of K over the 16 head groups (axis=heads)
    A = Q * (K / Kc) * V            (purely elementwise)
    out = A @ wc + bc

Everything is per-token, so we shard the 32768 tokens across the 8 cores
(4096 tokens each) with zero communication.  The host feeds each core its
token slice pre-transposed to feature-major [D, tok] (bf16) so the device
never has to transpose activations for the matmuls; intermediates are kept
token-major in SBUF.

Key layout trick: the host permutes the OUTPUT columns of wq/wk/wv (and the
rows of wc to match) so that projected features are ordered channel-major,
f' = dd*16 + h.  The 16 heads of one depth-channel are then 16 consecutive
elements of the free axis, and the whole head-axis cumsum collapses into a
single masked tensor_tensor_scan (segmented prefix sum) per token block —
no 15-deep add chain, and no special-casing of head 0.

The A^T needed by the output projection is produced by the DMA transpose
xbar (one descriptor per 128-token block), keeping the PE free for real
matmuls.  Elementwise work is spread across DVE (min/recip/mults), Act
(relu/exp/V copy) and Pool (elu adds, scan, output copy) so each engine
stays well below the Tensor engine's ~27 us/chunk.  Matmul operands are
bf16 with fp32 PSUM accumulation.
"""

import sys

sys.path.insert(0, "/opt/trn_rl_repo")

import numpy as np
import ml_dtypes

B, L, D_MODEL, N_HEADS = 4, 8192, 1024, 16
D_HEAD = D_MODEL // N_HEADS  # 64
N_CORES = 8
TOKENS = B * L  # 32768
TOK_PER_CORE = TOKENS // N_CORES  # 4096
P = 128
KO = D_MODEL // P  # 8 k-blocks
import os as _os

T_CHUNK = int(_os.environ.get("KERNEL_TCHUNK", "256"))  # tokens per chunk
N_CHUNKS = TOK_PER_CORE // T_CHUNK
MB = T_CHUNK // P  # token-blocks per chunk
N_HALF = 512  # matmul moving width (one fp32 PSUM bank)
NH = D_MODEL // N_HALF  # 2

_BF16 = ml_dtypes.bfloat16

# How A^T is produced for the output projection:
#   "dma" — DMA transpose xbar (0 PE cycles; cost model says ~0.9 us/chunk)
#   "pe"  — PE transpose via identity matmul + copy (adds ~0.85 us/chunk PE)
import os

TRANSPOSE_MODE = os.environ.get("KERNEL_TRANSPOSE", "dma")
# MERGED: one 2-bank PSUM tile per token block and full-width (1024)
# epilogue ops — half the elementwise instruction count.
MERGED = os.environ.get("KERNEL_MERGED", "0") == "1"

# feature permutation: new column f' = dd*16 + h  <-  old column h*64 + dd
_OLD_OF_NEW = np.array(
    [(f % N_HEADS) * D_HEAD + f // N_HEADS for f in range(D_MODEL)], dtype=np.int64
)

_module_cache = {}


def _build_module(with_bias: bool, repeat: int = 1):
    import contextlib

    import concourse.mybir as mybir
    import concourse.tile as tile
    from concourse import bacc
    from concourse.masks import make_identity

    f32 = mybir.dt.float32
    bf16 = mybir.dt.bfloat16
    AF = mybir.ActivationFunctionType
    OP = mybir.AluOpType
    pe_transpose = TRANSPOSE_MODE == "pe"

    nc = bacc.Bacc("TRN2", target_bir_lowering=False, debug=False)

    xq_d = nc.dram_tensor("xq_t", (D_MODEL, TOK_PER_CORE), bf16, kind="ExternalInput")
    xk_d = nc.dram_tensor("xk_t", (D_MODEL, TOK_PER_CORE), bf16, kind="ExternalInput")
    xv_d = nc.dram_tensor("xv_t", (D_MODEL, TOK_PER_CORE), bf16, kind="ExternalInput")
    w_d = {
        name: nc.dram_tensor(name, (D_MODEL, D_MODEL), bf16, kind="ExternalInput")
        for name in ("wq", "wk", "wv", "wc")
    }
    mask_d = nc.dram_tensor("scanmask", (P, D_MODEL), f32, kind="ExternalInput")
    out_d = nc.dram_tensor("out_t", (TOK_PER_CORE, D_MODEL), f32, kind="ExternalOutput")

    b_d = {}
    if with_bias:
        # replicated across partitions for token-major free-axis adds
        for name in ("bq", "bk", "bv", "bc"):
            b_d[name] = nc.dram_tensor(
                f"{name}_rep", (P, D_MODEL), f32, kind="ExternalInput"
            )

    xq_r = xq_d.rearrange("(ko p) t -> p ko t", p=P)
    xk_r = xk_d.rearrange("(ko p) t -> p ko t", p=P)
    xv_r = xv_d.rearrange("(ko p) t -> p ko t", p=P)

    with tile.TileContext(nc) as tc:
        with (
            tc.tile_pool(name="const", bufs=1) as constp,
            tc.tile_pool(name="xin", bufs=2) as xinp,
            tc.tile_pool(name="work", bufs=2) as workp,
            tc.tile_pool(name="small", bufs=4) as smallp,
            tc.tile_pool(name="kcpool", bufs=2) as kcp,
            tc.tile_pool(
                name="psum1",
                bufs=3 if MERGED else (4 if pe_transpose else 5),
                space="PSUM",
            ) as pp1,
            tc.tile_pool(
                name="psum2", bufs=1 if MERGED else (2 if pe_transpose else 3),
                space="PSUM",
            ) as pp2,
            tc.tile_pool(name="psumT", bufs=2, space="PSUM") as ppT,
        ):
            # Weights resident in SBUF; loaded on the gpsimd DMA queue so the
            # first chunk's activation loads (sync queue) are not stuck
            # behind 8 MB of weights in the same FIFO.
            w_sb = {}
            for name in ("wk", "wq", "wv", "wc"):
                t = constp.tile([P, KO, D_MODEL], bf16, tag=f"{name}_sb")
                w_r = w_d[name].rearrange("(ko p) n -> p ko n", p=P)
                if name == "wk":
                    # first weight used: one DMA per ko block, alternating
                    # queues, so the ko=0 matmul only waits ~1 us for its
                    # first 256 KB instead of 4 us for half the matrix
                    for ko in range(KO):
                        eng = nc.scalar if ko % 2 == 0 else nc.gpsimd
                        eng.dma_start(t[:, ko], w_r[:, ko])
                else:
                    nc.gpsimd.dma_start(t[:], w_r)
                w_sb[name] = t

            mask_sb = constp.tile([P, D_MODEL], f32, tag="mask_sb")
            nc.scalar.dma_start(mask_sb[:], mask_d[:])

            ident = None
            if pe_transpose:
                ident = constp.tile([P, P], bf16, tag="ident")
                make_identity(nc, ident)

            b_sb = {}
            if with_bias:
                for name in ("bq", "bk", "bv", "bc"):
                    t = constp.tile([P, D_MODEL], f32, tag=f"{name}_sb")
                    nc.gpsimd.dma_start(t[:], b_d[name][:])
                    b_sb[name] = t

            def proj_epilogue(src, dslice, mb, bias, elu, qv_with, width):
                """src: PSUM AP, dslice: bf16 SBUF dst of the same width."""
                if bias is not None:
                    tmp = smallp.tile([P, width], f32, tag="btmp")
                    nc.vector.tensor_tensor(tmp[:], src, bias, OP.add)
                    src = tmp[:]
                if elu:
                    # elu(x)+1 == relu(x) + exp(min(x, 0))  (exact).
                    # PSUM readers (min, relu) on DVE/Act — GPSIMD cannot
                    # access PSUM; the SBUF-only add goes to the otherwise
                    # idle Pool engine.
                    e = smallp.tile([P, width], bf16, tag="e")
                    nc.vector.tensor_scalar_min(e[:], src, 0.0)
                    nc.scalar.activation(dslice, src, AF.Relu)
                    nc.scalar.activation(e[:], e[:], AF.Exp)
                    nc.gpsimd.tensor_tensor(dslice, dslice, e[:], OP.add)
                elif qv_with is not None:
                    nc.vector.tensor_tensor(dslice, qv_with, src, OP.mult)
                else:
                    nc.scalar.activation(dslice, src, AF.Copy)

            def proj(x_t, w, dst, bias, elu=True, qv_with=None):
                """dst[:, mb, :] (token-major [P, MB, D] bf16) = act(x @ W + b).

                Loop order mb -> ko -> nh: the stationary operand (a 128x128
                token block of x^T) is reused for both n-halves; the nh
                halves interleave so consecutive matmuls target alternating
                PSUM banks and the PE pipeline never bubbles.

                With qv_with=Q (the V projection), the epilogue is instead
                the fused multiply dst = Q * (x @ W + b), reading the
                projection straight out of PSUM.
                """
                for mb in range(MB):
                    if MERGED:
                        # one 2-bank PSUM tile per mb: full-width epilogue
                        # ops (half the instruction count); each matmul still
                        # targets a single bank via its 512-column slice
                        ps = pp1.tile([P, D_MODEL], f32, tag="ps1", name=f"ps1_{mb}")
                        pss = [ps[:, i * N_HALF : (i + 1) * N_HALF] for i in range(NH)]
                    else:
                        pss = [
                            pp1.tile(
                                [P, N_HALF], f32, tag="ps1", name=f"ps1_{mb}_{i}"
                            )[:]
                            for i in range(NH)
                        ]
                    for ko in range(KO):
                        for nh in range(NH):
                            nc.tensor.matmul(
                                pss[nh],
                                lhsT=x_t[:, ko, mb * P : (mb + 1) * P],
                                rhs=w[:, ko, nh * N_HALF : (nh + 1) * N_HALF],
                                start=(ko == 0),
                                stop=(ko == KO - 1),
                            )
                    if MERGED:
                        proj_epilogue(
                            ps[:],
                            dst[:, mb, :],
                            mb,
                            bias[:] if bias is not None else None,
                            elu,
                            qv_with[:, mb, :] if qv_with is not None else None,
                            D_MODEL,
                        )
                    else:
                        for nh in range(NH):
                            nhs = slice(nh * N_HALF, (nh + 1) * N_HALF)
                            proj_epilogue(
                                pss[nh],
                                dst[:, mb, nhs],
                                mb,
                                bias[:, nhs] if bias is not None else None,
                                elu,
                                qv_with[:, mb, nhs] if qv_with is not None else None,
                                N_HALF,
                            )

            def ratio_block(K, mb):
                # Segmented prefix sum over the 16 heads: features are
                # channel-major (f' = dd*16 + h), so each depth channel's 16
                # heads are consecutive and one masked scan does the cumsum:
                #   state = mask[f]*state + K[f];  mask = 0 at f % 16 == 0.
                # (TensorTensorScanArith only exists on DVE.)  Issued after
                # the Q projection so the Q-PSUM-freeing min ops sit ahead
                # of this chain in the DVE queue.
                Kc = kcp.tile([P, D_MODEL], f32, tag="kc", name=f"kc_{mb}")
                nc.vector.tensor_tensor_scan(
                    Kc[:], mask_sb[:], K[:, mb, :], 0.0, OP.mult, OP.add
                )
                # Kc := 1/Kc  (Kc strictly positive: sums of elu(x)+1 > 0)
                nc.vector.reciprocal_approx_fast(Kc[:], Kc[:])
                # K := K * (1/Kc) — the per-head attention ratio
                nc.vector.tensor_tensor(K[:, mb], K[:, mb], Kc[:], OP.mult)

            def out_block(t0, AT, mb):
                """Output projection for one 128-token block: out = A @ wc,
                token-major: lhsT = feature-major A^T block (stationary,
                reused for both n-halves), rhs = wc rows."""
                if MERGED:
                    ps = pp2.tile([P, D_MODEL], f32, tag="ps2", name=f"ps2_{t0}_{mb}")
                    pss = [ps[:, i * N_HALF : (i + 1) * N_HALF] for i in range(NH)]
                else:
                    pss = [
                        pp2.tile(
                            [P, N_HALF], f32, tag="ps2", name=f"ps2_{t0}_{mb}_{i}"
                        )[:]
                        for i in range(NH)
                    ]
                for ko in range(KO):
                    for nh in range(NH):
                        nc.tensor.matmul(
                            pss[nh],
                            lhsT=AT[:, ko, mb * P : (mb + 1) * P],
                            rhs=w_sb["wc"][:, ko, nh * N_HALF : (nh + 1) * N_HALF],
                            start=(ko == 0),
                            stop=(ko == KO - 1),
                        )
                srcs = (
                    [(ps[:], slice(0, D_MODEL), D_MODEL)]
                    if MERGED
                    else [
                        (pss[nh], slice(nh * N_HALF, (nh + 1) * N_HALF), N_HALF)
                        for nh in range(NH)
                    ]
                )
                for src, nhs, width in srcs:
                    ot = smallp.tile([P, width], f32, tag="osb")
                    if with_bias:
                        nc.scalar.activation(ot[:], src, AF.Copy)
                        nc.vector.tensor_tensor(
                            ot[:], ot[:], b_sb["bc"][:, nhs], OP.add
                        )
                    else:
                        nc.scalar.activation(ot[:], src, AF.Copy)
                    nc.scalar.dma_start(
                        out_d[t0 + mb * P : t0 + (mb + 1) * P, nhs], ot[:]
                    )

            def pe_transpose_block(A, AT):
                """A -> AT on the PE via identity matmuls; runs a full chunk
                after A was produced, so it never waits on the elementwise
                chain.  PSUM->SBUF copies alternate DVE/Act."""
                i = 0
                for fb in range(KO):
                    for mb in range(MB):
                        pt = ppT.tile([P, P], bf16, tag="pT")
                        nc.tensor.transpose(
                            pt[:], A[:, mb, fb * P : (fb + 1) * P], ident[:]
                        )
                        if i % 2 == 0:
                            nc.vector.tensor_copy(
                                AT[:, fb, mb * P : (mb + 1) * P], pt[:]
                            )
                        else:
                            nc.scalar.activation(
                                AT[:, fb, mb * P : (mb + 1) * P], pt[:], AF.Copy
                            )
                        i += 1

            def chunk_body(c, pending):
                """Projections + elementwise for chunk c; the output
                projection for chunk c-1 (`pending`) is interleaved before
                the V projection, so its A^T operand was produced a full
                chunk earlier and the PE never waits on the elementwise
                chain."""
                t0 = c * T_CHUNK
                xk_t = xinp.tile([P, KO, T_CHUNK], bf16, tag="xk")
                nc.sync.dma_start(xk_t[:], xk_r[:, :, t0 : t0 + T_CHUNK])
                xq_t = xinp.tile([P, KO, T_CHUNK], bf16, tag="xq")
                nc.sync.dma_start(xq_t[:], xq_r[:, :, t0 : t0 + T_CHUNK])
                xv_t = xinp.tile([P, KO, T_CHUNK], bf16, tag="xv")
                nc.sync.dma_start(xv_t[:], xv_r[:, :, t0 : t0 + T_CHUNK])

                Q = workp.tile([P, MB, D_MODEL], bf16, tag="Q")
                K = workp.tile([P, MB, D_MODEL], bf16, tag="K")
                A = workp.tile([P, MB, D_MODEL], bf16, tag="A")
                AT = workp.tile([P, KO, MB * P], bf16, tag="AT")

                # K first: the scan -> recip -> ratio chain (DVE, issued
                # after Q's epilogue) completes while the V projection and
                # the previous chunk's output projection occupy the PE.
                proj(xk_t, w_sb["wk"], K, b_sb.get("bk"))
                proj(xq_t, w_sb["wq"], Q, b_sb.get("bq"))
                for mb in range(MB):
                    ratio_block(K, mb)

                # Previous chunk's transpose + output projection BEFORE the
                # V projection: the PSUM->SBUF copies then clear the engine
                # queues mid-chunk instead of blocking the next chunk's
                # epilogues at the boundary.
                if pending is not None:
                    pt0, pA, pAT = pending
                    if pe_transpose:
                        pe_transpose_block(pA, pAT)
                    for mb in range(MB):
                        out_block(pt0, pAT, mb)

                # V projection epilogue writes A = Q * V straight from PSUM
                proj(xv_t, w_sb["wv"], A, b_sb.get("bv"), elu=False, qv_with=Q)

                for mb in range(MB):
                    # A = (Q*V) * ratio
                    nc.vector.tensor_tensor(A[:, mb], A[:, mb], K[:, mb], OP.mult)
                    if not pe_transpose:
                        # feature-major via the DMA xbar (0 PE cycles)
                        nc.sync.dma_start_transpose(
                            AT[:, :, mb * P : (mb + 1) * P], A[:, mb, :]
                        )
                return (t0, A, AT)

            repeat_ctx = (
                tc.For_i(0, repeat, 1) if repeat > 1 else contextlib.nullcontext()
            )
            with repeat_ctx:
                pending = None
                for c in range(N_CHUNKS):
                    pending = chunk_body(c, pending)
                pt0, pA, pAT = pending
                if pe_transpose:
                    pe_transpose_block(pA, pAT)
                for mb in range(MB):
                    out_block(pt0, pAT, mb)

    nc.compile()
    return nc


def _get_module(with_bias: bool, repeat: int = 1):
    key = (bool(with_bias), repeat, TRANSPOSE_MODE)
    if key not in _module_cache:
        _module_cache[key] = _build_module(bool(with_bias), repeat)
    return _module_cache[key]


def _prepare_in_maps(v, k, q, wq_w, wq_b, wk_w, wk_b, wv_w, wv_b, wc_w, wc_b):
    with_bias = any(np.any(np.asarray(b)) for b in (wq_b, wk_b, wv_b, wc_b))

    q2 = np.asarray(q, dtype=np.float32).reshape(TOKENS, D_MODEL)
    k2 = np.asarray(k, dtype=np.float32).reshape(TOKENS, D_MODEL)
    v2 = np.asarray(v, dtype=np.float32).reshape(TOKENS, D_MODEL)

    # channel-major feature permutation (see module docstring)
    w16 = {
        "wq": np.ascontiguousarray(
            np.asarray(wq_w, np.float32)[:, _OLD_OF_NEW]
        ).astype(_BF16),
        "wk": np.ascontiguousarray(
            np.asarray(wk_w, np.float32)[:, _OLD_OF_NEW]
        ).astype(_BF16),
        "wv": np.ascontiguousarray(
            np.asarray(wv_w, np.float32)[:, _OLD_OF_NEW]
        ).astype(_BF16),
        "wc": np.ascontiguousarray(
            np.asarray(wc_w, np.float32)[_OLD_OF_NEW, :]
        ).astype(_BF16),
    }

    maskrow = np.where(np.arange(D_MODEL) % N_HEADS == 0, 0.0, 1.0).astype(np.float32)
    mask = np.ascontiguousarray(np.broadcast_to(maskrow, (P, D_MODEL)))

    bias_maps = {}
    if with_bias:
        bias_maps = {
            "bq_rep": np.ascontiguousarray(
                np.broadcast_to(
                    np.asarray(wq_b, np.float32)[_OLD_OF_NEW], (P, D_MODEL)
                )
            ),
            "bk_rep": np.ascontiguousarray(
                np.broadcast_to(
                    np.asarray(wk_b, np.float32)[_OLD_OF_NEW], (P, D_MODEL)
                )
            ),
            "bv_rep": np.ascontiguousarray(
                np.broadcast_to(
                    np.asarray(wv_b, np.float32)[_OLD_OF_NEW], (P, D_MODEL)
                )
            ),
            "bc_rep": np.ascontiguousarray(
                np.broadcast_to(np.asarray(wc_b, np.float32), (P, D_MODEL))
            ),
        }

    in_maps = []
    for c in range(N_CORES):
        s = slice(c * TOK_PER_CORE, (c + 1) * TOK_PER_CORE)
        m = {
            "xq_t": np.ascontiguousarray(q2[s].T).astype(_BF16),
            "xk_t": np.ascontiguousarray(k2[s].T).astype(_BF16),
            "xv_t": np.ascontiguousarray(v2[s].T).astype(_BF16),
            "scanmask": mask,
            **w16,
            **bias_maps,
        }
        in_maps.append(m)
    return in_maps, with_bias


def _assemble(results):
    out = np.empty((TOKENS, D_MODEL), np.float32)
    for c in range(N_CORES):
        out[c * TOK_PER_CORE : (c + 1) * TOK_PER_CORE] = results[c]["out_t"]
    return out.reshape(B, L, D_MODEL)


def run_kernel_raw(trace=False, **inputs):
    """Run on the 8 NeuronCores; returns (output, BassKernelResults)."""
    from concourse.bass_utils import run_bass_kernel_spmd

    in_maps, with_bias = _prepare_in_maps(**inputs)
    nc = _get_module(with_bias)
    res = run_bass_kernel_spmd(nc, in_maps, core_ids=list(range(N_CORES)), trace=trace)
    return _assemble(res.results), res


def kernel(**inputs):
    out, _ = run_kernel_raw(trace=False, **inputs)
    return out
